# revision 1
# baseline (speedup 1.0000x reference)
"""3-layer GCN on 8 TRN2 NeuronCores.

Strategy (edge-cut / dst-partitioned, per sharding hint):
- Each core owns 12500 dst nodes (12544 padded slots). Host routes each edge
  to the core owning its dst.
- Per layer: bulk dma_gather pulls h[src] rows (256B pair-elements, 4 classes
  by src parity/bank for int16 index range), dma_scatter_add accumulates them
  into a per-core agg table in DRAM (edges grouped so each scatter call has
  distinct dst slots; calls serialize on the agg tensor).
- Dense part on PE: z = agg @ W + b (+ LeakyReLU), written into an internal
  DRAM tensor; AllGather collective rebuilds the full node table for the next
  layer's gather.
- Layer 3 output slices are returned per core and reassembled on host.
"""

import json
import os

import numpy as np

import concourse.bacc as bacc
import concourse.bass as bass
import concourse.mybir as mybir
import concourse.tile as tile

N = 100000
E_TOT = 1600000
NC = 8
OWN = 12500  # real dst nodes per core
S = 12544  # padded slots per core (98 * 128)
NQ = S // 128  # 98 agg chunks
F = 32
FO_L = [32, 32, 16]
BANK = 25088  # pair-rows per bank
CHUNK = 2048  # max edges per dma_gather/scatter call (walrus 16-bit sem limit)
DEAD_SLOT = OWN  # scatter pad target
XV_ROWS = 50177  # x view [XV_ROWS, 64] = 100354 padded rows of 32
ZV_COUNT = [25088, 25088, 25087, 25087]  # per-class AP row counts for z views
XV_COUNT = [25088, 25088, 25088, 25088]

_cache = {}


# ---------------------------------------------------------------- BIR patch
def _split_sync_waits(bir_json, max_waits=1):
    d = json.loads(bir_json.decode() if isinstance(bir_json, (bytes, bytearray)) else bir_json)
    ctr = 0
    for f in d.get("functions", []):
        for bb in f.get("blocks", []):
            insts = bb.get("instructions", [])
            if not any(
                len((i.get("sync_info") or {}).get("on_wait") or []) > max_waits
                for i in insts
            ):
                continue
            out = []
            for inst in insts:
                si = inst.get("sync_info")
                waits = (si or {}).get("on_wait") or []
                if len(waits) > max_waits:
                    extra = waits[: len(waits) - max_waits]
                    si["on_wait"] = waits[len(waits) - max_waits :]
                    for w in extra:
                        ctr += 1
                        out.append(
                            {
                                "debug": inst.get("debug", 0),
                                "engine": inst["engine"],
                                "ins": [],
                                "outs": [],
                                "name": f"waitsplit-{ctr}",
                                "opcode": "NoOp",
                                "sync_info": {"on_update": [], "on_wait": [w]},
                            }
                        )
                out.append(inst)
            bb["instructions"] = out
    return json.dumps(d).encode()


_patched = False


def _install_birpatch():
    global _patched
    if _patched:
        return
    _patched = True
    import concourse.bass_utils as bu

    orig = bu.compile_bir_kernel

    def patched(bir_json, tmpdir, neff_name="file.neff"):
        return orig(_split_sync_waits(bir_json), tmpdir, neff_name=neff_name)

    bu.compile_bir_kernel = patched
    try:
        import concourse.bass2jax as b2j

        b2j.compile_bir_kernel = patched
    except ImportError:
        pass


# ------------------------------------------------------------- host planning
def _round_up(x, m):
    return ((x + m - 1) // m) * m


def _build_plan(src_g, slots, cores):
    """Group edges of each core into (class, group) segments.

    src_g: global gather address per edge (node id in the source table's
    row-of-32 numbering). slots: dst slot per edge. cores: owning core.
    Returns per-core index arrays and the shared static call structure.
    """
    cls_all = (src_g % 2) + 2 * ((src_g // 2) // BANK)
    per_core = []
    counts = np.zeros((NC, 4, 64), np.int64)  # generous G cap
    for c in range(NC):
        m = cores == c
        s_g = src_g[m]
        sl = slots[m]
        cl = cls_all[m]
        # rank of each edge within its (class, slot) group
        order = np.lexsort((sl, cl))
        cl_s, sl_s, g_s = cl[order], sl[order], s_g[order]
        key = cl_s.astype(np.int64) * S + sl_s
        first = np.r_[True, key[1:] != key[:-1]]
        grp_start = np.flatnonzero(first)
        rank = np.arange(len(key)) - np.repeat(grp_start, np.diff(np.r_[grp_start, len(key)]))
        for cls in range(4):
            mm = cl_s == cls
            if mm.any():
                bc = np.bincount(rank[mm], minlength=64)
                counts[c, cls, : len(bc)] += bc
        per_core.append((cl_s, sl_s, g_s, rank))
    cap = counts.max(axis=0)  # [4, 64]
    G = int(np.max(np.nonzero(cap.sum(axis=0))[0])) + 1 if cap.sum() else 1
    cap = cap[:, :G]
    cap = np.maximum(_round_up(cap, 128), 0)
    # segment offsets: class-major, then group
    seg_off = np.zeros((4, G), np.int64)
    cls_base = np.zeros(5, np.int64)
    off = 0
    for cls in range(4):
        cls_base[cls] = off
        for g in range(G):
            seg_off[cls, g] = off
            off += cap[cls, g]
    cls_base[4] = off
    T = off

    # static call structure
    gather_calls = []  # (cls, start, n)
    for cls in range(4):
        a = cls_base[cls]
        while a < cls_base[cls + 1]:
            n = min(CHUNK, cls_base[cls + 1] - a)
            gather_calls.append((cls, int(a), int(n)))
            a += n
    scatter_calls = []  # (gather_call_idx, col_a, col_b)
    for gi, (cls, ga, gn) in enumerate(gather_calls):
        for g in range(G):
            s0 = max(seg_off[cls, g], ga)
            s1 = min(seg_off[cls, g] + cap[cls, g], ga + gn)
            if s1 > s0:
                scatter_calls.append((gi, int((s0 - ga) // 128), int((s1 - ga) // 128)))

    # per-core index arrays
    gidx = np.zeros((NC, T), np.int16)
    sidx = np.full((NC, T), DEAD_SLOT, np.int16)
    for c in range(NC):
        cl_s, sl_s, g_s, rank = per_core[c]
        pos = seg_off[cl_s, rank] + _within_seg_pos(cl_s, rank, G)
        bank = (g_s // 2) // BANK
        gidx_val = (g_s // 2) - bank * BANK
        gidx[c, pos] = gidx_val.astype(np.int16)
        sidx[c, pos] = sl_s.astype(np.int16)
    return {
        "T": int(T),
        "gather_calls": gather_calls,
        "scatter_calls": scatter_calls,
        "gidx": gidx,
        "sidx": sidx,
    }


def _within_seg_pos(cl_s, rank, G):
    """Sequential position of each edge within its (class, group) segment.

    Edges are already sorted by (class, slot) and rank is the per-slot rank,
    so within a segment the slot order is increasing; enumerate occurrences.
    """
    key = cl_s.astype(np.int64) * G + rank
    order = np.argsort(key, kind="stable")
    inv = np.empty_like(order)
    inv[order] = np.arange(len(order))
    ks = key[order]
    first = np.r_[True, ks[1:] != ks[:-1]]
    seg_start = np.flatnonzero(first)
    within_sorted = np.arange(len(ks)) - np.repeat(
        seg_start, np.diff(np.r_[seg_start, len(ks)])
    )
    return within_sorted[inv]


def _wrap_idx(arr):
    """[T] -> [128, T/16] int16, index i at [i%16 (replicated x8), i//16]."""
    T = arr.shape[0]
    w = arr.reshape(T // 16, 16).T  # [16, T/16]
    return np.tile(w, (8, 1)).copy()


# --------------------------------------------------------------- bass build
def _build_nc(plan_x, plan_z):
    nc = bacc.Bacc("TRN2", target_bir_lowering=False, debug=False, num_devices=NC)
    f32, i16 = mybir.dt.float32, mybir.dt.int16

    xv = nc.dram_tensor("xv", [XV_ROWS, 64], f32, kind="ExternalInput")
    w_in = [
        nc.dram_tensor(f"w{i}", [F, FO_L[i]], f32, kind="ExternalInput")
        for i in range(3)
    ]
    b_in = [
        nc.dram_tensor(f"b{i}", [128, FO_L[i]], f32, kind="ExternalInput")
        for i in range(3)
    ]
    gidx_in = [
        nc.dram_tensor("gidx_x", [128, plan_x["T"] // 16], i16, kind="ExternalInput"),
        nc.dram_tensor("gidx_z", [128, plan_z["T"] // 16], i16, kind="ExternalInput"),
    ]
    sidx_in = [
        nc.dram_tensor("sidx_x", [128, plan_x["T"] // 16], i16, kind="ExternalInput"),
        nc.dram_tensor("sidx_z", [128, plan_z["T"] // 16], i16, kind="ExternalInput"),
    ]
    out = nc.dram_tensor("out", [S, FO_L[2]], f32, kind="ExternalOutput")

    agg = nc.dram_tensor("agg", [S, 64], f32, kind="Internal")
    cc_in = [
        nc.dram_tensor(f"cc_in{i}", [S, F], f32, kind="Internal") for i in range(2)
    ]
    cc_out = [
        nc.dram_tensor(f"cc_out{i}", [NC * S, F], f32, kind="Internal", addr_space="Shared")
        for i in range(2)
    ]

    def src_ap(layer, cls):
        if layer == 0:
            t, cnt = xv[:].tensor, XV_COUNT[cls]
        else:
            t, cnt = cc_out[layer - 1][:].tensor, ZV_COUNT[cls]
        bank, par = cls // 2, cls % 2
        off = bank * BANK * 64 + par * 32
        return bass.AP(t, off, [[64, cnt], [1, 64]])

    agg_pview = agg[:].rearrange("(q p) f -> p q f", p=128)
    agg_qview = agg[:].rearrange("(q p) f -> q p f", p=128)

    with tile.TileContext(nc) as tc:
        with (
            tc.tile_pool(name="consts", bufs=1) as constp,
            tc.tile_pool(name="idx", bufs=2) as idxp,
            tc.tile_pool(name="gat", bufs=4) as gatp,
            tc.tile_pool(name="zst", bufs=4) as zstp,
            tc.tile_pool(name="psum", bufs=4, space="PSUM") as psump,
        ):
            w_t = []
            b_t = []
            for i in range(3):
                wt = constp.tile([F, FO_L[i]], f32, tag=f"w{i}")
                bt = constp.tile([128, FO_L[i]], f32, tag=f"b{i}")
                nc.sync.dma_start(wt[:], w_in[i][:])
                nc.sync.dma_start(bt[:], b_in[i][:])
                w_t.append(wt)
                b_t.append(bt)
            zero_t = constp.tile([128, 3136], f32, tag="zero")
            nc.vector.memset(zero_t[:], 0.0)

            for layer in range(3):
                plan = plan_x if layer == 0 else plan_z
                pi = 0 if layer == 0 else 1
                fo = FO_L[layer]

                # load index arrays (layer 1: x plan; layer 2: z plan; layer 3 reuses)
                if layer <= 1:
                    gidx_t = idxp.tile([128, plan["T"] // 16], i16, tag="gidx")
                    sidx_t = idxp.tile([128, plan["T"] // 16], i16, tag="sidx")
                    nc.sync.dma_start(gidx_t[:], gidx_in[pi][:])
                    nc.sync.dma_start(sidx_t[:], sidx_in[pi][:])

                # zero agg
                z3v = zero_t[:].rearrange("p (q f) -> p q f", f=64)
                nc.sync.dma_start(agg_pview[:, :49, :], z3v)
                nc.sync.dma_start(agg_pview[:, 49:, :], z3v)

                # gather + scatter
                g_tiles = {}
                for gi, (cls, ga, gn) in enumerate(plan["gather_calls"]):
                    g = gatp.tile([128, (CHUNK // 128) * 64], f32, tag="g")
                    g3 = g[:, : (gn // 128) * 64].rearrange("p (k f) -> p k f", f=64)
                    nc.gpsimd.dma_gather(
                        out_ap=g3,
                        in_ap=src_ap(layer, cls),
                        idxs_ap=gidx_t[:, ga // 16 : (ga + gn) // 16],
                        num_idxs=gn,
                        num_idxs_reg=gn,
                        elem_size=64,
                        single_packet=False,
                    )
                    g_tiles[gi] = (g, ga, gn)
                    for gi2, ca, cb in plan["scatter_calls"]:
                        if gi2 != gi:
                            continue
                        nn = (cb - ca) * 128
                        nc.gpsimd.dma_scatter_add(
                            out_ap=agg[:],
                            in_ap=g[:, ca * 64 : cb * 64].rearrange(
                                "p (k f) -> p k f", f=64
                            ),
                            idxs_ap=sidx_t[
                                :, (ga + ca * 128) // 16 : (ga + cb * 128) // 16
                            ],
                            num_idxs=nn,
                            num_idxs_reg=nn,
                            elem_size=64,
                            single_packet=False,
                        )

                # dense stage: z = agg[:, :32] @ W + b (+ leaky relu)
                for q in range(NQ):
                    t = zstp.tile([128, 64], f32, tag="aggtile")
                    nc.sync.dma_start(t[:], agg_qview[q])
                    at = zstp.tile([32, 128], f32, tag="aggT")
                    for k in range(4):
                        nc.vector.transpose(
                            at[:, 32 * k : 32 * k + 32], t[32 * k : 32 * k + 32, :32]
                        )
                    pz = psump.tile([128, fo], f32, tag="pz")
                    nc.tensor.matmul(pz[:], lhsT=at[:], rhs=w_t[layer][:], start=True, stop=True)
                    zz = zstp.tile([128, fo], f32, tag="zz")
                    nc.vector.tensor_tensor(
                        out=zz[:], in0=pz[:], in1=b_t[layer][:], op=mybir.AluOpType.add
                    )
                    if layer < 2:
                        zm = zstp.tile([128, fo], f32, tag="zm")
                        nc.vector.tensor_scalar_mul(zm[:], zz[:], 0.1)
                        nc.vector.tensor_tensor(
                            out=zz[:], in0=zz[:], in1=zm[:], op=mybir.AluOpType.max
                        )
                        nc.sync.dma_start(
                            cc_in[layer][q * 128 : (q + 1) * 128, :], zz[:]
                        )
                    else:
                        nc.sync.dma_start(out[q * 128 : (q + 1) * 128, :], zz[:])

                if layer < 2:
                    nc.gpsimd.collective_compute(
                        "AllGather",
                        mybir.AluOpType.bypass,
                        ins=[cc_in[layer][:]],
                        outs=[cc_out[layer][:]],
                        replica_groups=[list(range(NC))],
                    )
    nc.compile()
    return nc


# ------------------------------------------------------------------- driver
def kernel(**inputs):
    _install_birpatch()
    x = np.asarray(inputs["x"], np.float32)
    src = np.asarray(inputs["src"], np.int64)
    dst = np.asarray(inputs["dst"], np.int64)
    Ws = [np.asarray(inputs[k], np.float32) for k in ("W1", "W2", "W3")]
    bs = [np.asarray(inputs[k], np.float32) for k in ("b1", "b2", "b3")]

    key = hash((src.tobytes(), dst.tobytes()))
    if key not in _cache:
        owner = dst // OWN
        slot = dst - owner * OWN
        # layer-1 addressing: x row order; layers 2-3: slot order
        src_owner = src // OWN
        src_slot_g = src_owner * S + (src - src_owner * OWN)
        plan_x = _build_plan(src, slot, owner)
        plan_z = _build_plan(src_slot_g, slot, owner)
        nc = _build_nc(plan_x, plan_z)
        _cache[key] = (nc, plan_x, plan_z, owner, slot)
    nc, plan_x, plan_z, owner, slot = _cache[key]

    xpad = np.zeros((XV_ROWS * 2, F), np.float32)
    xpad[:N] = x
    xv = xpad.reshape(XV_ROWS, 64)

    in_maps = []
    for c in range(NC):
        m = {
            "xv": xv,
            "gidx_x": _wrap_idx(plan_x["gidx"][c]),
            "sidx_x": _wrap_idx(plan_x["sidx"][c]),
            "gidx_z": _wrap_idx(plan_z["gidx"][c]),
            "sidx_z": _wrap_idx(plan_z["sidx"][c]),
        }
        for i in range(3):
            m[f"w{i}"] = Ws[i]
            m[f"b{i}"] = np.tile(bs[i][None, :], (128, 1))
        in_maps.append(m)

    from concourse.bass_utils import run_bass_kernel_spmd

    trace = os.environ.get("GCN_TRACE") == "1"
    res = run_bass_kernel_spmd(nc, in_maps, core_ids=list(range(NC)), trace=trace)
    global last_exec_ns
    last_exec_ns = res.exec_time_ns

    out = np.zeros((N, FO_L[2]), np.float32)
    for c in range(NC):
        z = res.results[c]["out"]  # [S, 16]
        out[c * OWN : (c + 1) * OWN] = z[:OWN]
    return out



# revision 3
# speedup vs baseline: 1.7987x; 1.7987x over previous
"""3-layer GCN on 8 TRN2 NeuronCores — scatter-free quad-gather design.

Each core owns 12500 dst nodes. Per layer, aggregation is a pure gather:
- dst slots are degree-sorted into 98 tiles of 128; tile t has D_t neighbor
  columns (shared across cores = max over cores).
- One dma_gather stream of 512B quad-row elements (4 feature rows per
  element; quad ids < 32768 fit int16 with no bank classes). Element for
  stream position p lands at [p%128, p//128] — position encodes
  (tile, column, node).
- A host-precomputed one-hot mask (streamed from DRAM, zero for pad cells,
  1.0 on byte-band src%4 for real edges) is multiplied in, then contiguous
  tree-folds reduce columns + quad bands to the [128, 32] aggregation tile.
- Dense stage: transpose + matmul + bias (+LeakyReLU) as usual; AllGather
  rebuilds the full node table between layers.

This removes all dma_scatter_add calls (previously ~45% of the serial
GpSimd descriptor-generation time) and all agg-table zeroing/RMW.
"""

import json
import os

import numpy as np

import concourse.bacc as bacc
import concourse.bass as bass
import concourse.mybir as mybir
import concourse.tile as tile

N = 100000
E_TOT = 1600000
NC = 8
OWN = 12500  # real dst nodes per core
S = 12544  # padded slots per core (98 * 128)
NT = S // 128  # 98 dst tiles
F = 32
FO_L = [32, 32, 16]
CHUNK = 2048  # edges per dma_gather call
XQ = 25000  # x quad rows (100000 / 4)
ZQ = (NC * S) // 4  # z quad rows

_cache = {}


# ---------------------------------------------------------------- BIR patch
def _split_sync_waits(bir_json, max_waits=1):
    d = json.loads(bir_json.decode() if isinstance(bir_json, (bytes, bytearray)) else bir_json)
    ctr = 0
    for f in d.get("functions", []):
        for bb in f.get("blocks", []):
            insts = bb.get("instructions", [])
            if not any(
                len((i.get("sync_info") or {}).get("on_wait") or []) > max_waits
                for i in insts
            ):
                continue
            out = []
            for inst in insts:
                si = inst.get("sync_info")
                waits = (si or {}).get("on_wait") or []
                if len(waits) > max_waits:
                    extra = waits[: len(waits) - max_waits]
                    si["on_wait"] = waits[len(waits) - max_waits :]
                    for w in extra:
                        ctr += 1
                        out.append(
                            {
                                "debug": inst.get("debug", 0),
                                "engine": inst["engine"],
                                "ins": [],
                                "outs": [],
                                "name": f"waitsplit-{ctr}",
                                "opcode": "NoOp",
                                "sync_info": {"on_update": [], "on_wait": [w]},
                            }
                        )
                out.append(inst)
            bb["instructions"] = out
    return json.dumps(d).encode()


_patched = False


def _install_birpatch():
    global _patched
    if _patched:
        return
    _patched = True
    import concourse.bass_utils as bu

    orig = bu.compile_bir_kernel

    def patched(bir_json, tmpdir, neff_name="file.neff"):
        return orig(_split_sync_waits(bir_json), tmpdir, neff_name=neff_name)

    bu.compile_bir_kernel = patched
    try:
        import concourse.bass2jax as b2j

        b2j.compile_bir_kernel = patched
    except ImportError:
        pass


# ------------------------------------------------------------- host planning
def _wrap_idx(arr):
    """[T] -> [128, T/16] int16, index i at [i%16 (replicated x8), i//16]."""
    T = arr.shape[0]
    w = arr.reshape(T // 16, 16).T  # [16, T/16]
    return np.tile(w, (8, 1)).copy()


def _rank_within_group(keys):
    """For each element, its occurrence rank among equal keys (keys arbitrary)."""
    order = np.argsort(keys, kind="stable")
    ks = keys[order]
    first = np.r_[True, ks[1:] != ks[:-1]]
    seg_start = np.flatnonzero(first)
    within = np.arange(len(ks)) - np.repeat(
        seg_start, np.diff(np.r_[seg_start, len(ks)])
    )
    rank = np.empty_like(within)
    rank[order] = within
    return rank


def _build_plan(src, dst):
    """Degree-sorted positional plan shared across cores + per-core arrays."""
    owner = dst // OWN
    # per-core degree sort
    orders = []  # core -> array [12500] node-local ids in position order
    pos_of = np.empty(N, np.int64)  # node -> global z-row position (core*S + pos)
    deg_sorted = np.zeros((NC, OWN), np.int64)
    for c in range(NC):
        m = owner == c
        dl = dst[m] - c * OWN
        deg = np.bincount(dl, minlength=OWN)
        order = np.argsort(-deg, kind="stable")
        orders.append(order)
        inv = np.empty(OWN, np.int64)
        inv[order] = np.arange(OWN)
        pos_of[c * OWN : (c + 1) * OWN] = c * S + inv
        deg_sorted[c] = deg[order]

    # shared per-tile column counts: max over cores of tile-max degree
    d_t = np.zeros(NT, np.int64)
    for t in range(NT):
        lo = t * 128
        hi = min(lo + 128, OWN)
        if lo >= OWN:
            break
        d_t[t] = max(int(deg_sorted[c][lo]) for c in range(NC))  # sorted desc
    colbase = np.zeros(NT + 1, np.int64)
    colbase[1:] = np.cumsum(d_t)
    ncols = int(colbase[NT])
    T = ncols * 128

    # static call/segment structure
    calls = []  # (start, n, [(tile, col_lo_local, col_hi_local), ...])
    a = 0
    while a < T:
        n = min(CHUNK, T - a)
        c0 = a // 128
        c1 = (a + n) // 128
        segs = []
        for t in range(NT):
            lo = max(colbase[t], c0)
            hi = min(colbase[t + 1], c1)
            if hi > lo:
                segs.append((t, int(lo - c0), int(hi - c0)))
        calls.append((int(a), int(n), segs))
        a += n

    # per-core index + mask arrays
    per_core = []
    zrow = pos_of  # node -> z-table row
    for c in range(NC):
        m = owner == c
        e_src = src[m]
        dl = dst[m] - c * OWN
        inv = np.empty(OWN, np.int64)
        inv[orders[c]] = np.arange(OWN)
        pos = inv[dl]  # 0..12499
        tl = pos // 128
        i = pos % 128
        j = _rank_within_group(pos)
        p_e = (colbase[tl] + j) * 128 + i
        assert (j < d_t[tl]).all()

        gx = np.zeros(T, np.int16)
        gz = np.zeros(T, np.int16)
        gx[p_e] = (e_src // 4).astype(np.int16)
        zr = zrow[e_src]
        gz[p_e] = (zr // 4).astype(np.int16)

        mx = np.zeros((T, 128), np.float32)
        mz = np.zeros((T, 128), np.float32)
        colx = (32 * (e_src % 4))[:, None] + np.arange(32)[None, :]
        colz = (32 * (zr % 4))[:, None] + np.arange(32)[None, :]
        mx[p_e[:, None], colx] = 1.0
        mz[p_e[:, None], colz] = 1.0
        # reshape to [128, ncols*128]: partition = stream i, free = (col, band)
        mx = mx.reshape(ncols, 128, 128).transpose(1, 0, 2).reshape(128, -1).copy()
        mz = mz.reshape(ncols, 128, 128).transpose(1, 0, 2).reshape(128, -1).copy()
        per_core.append(
            {
                "gx": _wrap_idx(gx),
                "gz": _wrap_idx(gz),
                "mx": mx,
                "mz": mz,
                "order": orders[c],
            }
        )
    return {"T": T, "ncols": ncols, "calls": calls, "colbase": colbase}, per_core


# --------------------------------------------------------------- bass build
def _build_nc(plan):
    nc = bacc.Bacc("TRN2", target_bir_lowering=False, debug=False, num_devices=NC)
    f32, i16 = mybir.dt.float32, mybir.dt.int16
    T, ncols = plan["T"], plan["ncols"]

    xq = nc.dram_tensor("xq", [XQ, 128], f32, kind="ExternalInput")
    w_in = [
        nc.dram_tensor(f"w{i}", [F, FO_L[i]], f32, kind="ExternalInput")
        for i in range(3)
    ]
    b_in = [
        nc.dram_tensor(f"b{i}", [128, FO_L[i]], f32, kind="ExternalInput")
        for i in range(3)
    ]
    gidx_in = [
        nc.dram_tensor("gx", [128, T // 16], i16, kind="ExternalInput"),
        nc.dram_tensor("gz", [128, T // 16], i16, kind="ExternalInput"),
    ]
    mask_in = [
        nc.dram_tensor("mx", [128, ncols * 128], f32, kind="ExternalInput"),
        nc.dram_tensor("mz", [128, ncols * 128], f32, kind="ExternalInput"),
    ]
    out = nc.dram_tensor("out", [S, FO_L[2]], f32, kind="ExternalOutput")

    cc_in = [
        nc.dram_tensor(f"cc_in{i}", [S, F], f32, kind="Internal") for i in range(2)
    ]
    cc_out = [
        nc.dram_tensor(f"cc_out{i}", [NC * S, F], f32, kind="Internal", addr_space="Shared")
        for i in range(2)
    ]

    def src_ap(layer):
        if layer == 0:
            return bass.AP(xq[:].tensor, 0, [[128, XQ], [1, 128]])
        t = cc_out[layer - 1][:].tensor
        return bass.AP(t, 0, [[128, ZQ], [1, 128]])

    with tile.TileContext(nc) as tc:
        with (
            tc.tile_pool(name="consts", bufs=1) as constp,
            tc.tile_pool(name="idx", bufs=1) as idxp,
            tc.tile_pool(name="acc", bufs=1) as accp,
            tc.tile_pool(name="gat", bufs=3) as gatp,
            tc.tile_pool(name="msk", bufs=3) as mskp,
            tc.tile_pool(name="zst", bufs=4) as zstp,
            tc.tile_pool(name="psum", bufs=4, space="PSUM") as psump,
        ):
            w_t = []
            b_t = []
            for i in range(3):
                wt = constp.tile([F, FO_L[i]], f32, tag=f"w{i}")
                bt = constp.tile([128, FO_L[i]], f32, tag=f"b{i}")
                nc.sync.dma_start(wt[:], w_in[i][:])
                nc.sync.dma_start(bt[:], b_in[i][:])
                w_t.append(wt)
                b_t.append(bt)

            gidx_t = [
                idxp.tile([128, T // 16], i16, tag="gx", name="gx_t"),
                idxp.tile([128, T // 16], i16, tag="gz", name="gz_t"),
            ]
            nc.sync.dma_start(gidx_t[0][:], gidx_in[0][:])
            nc.sync.dma_start(gidx_t[1][:], gidx_in[1][:])

            acc_t = accp.tile([128, NT * F], f32, tag="acc")

            for layer in range(3):
                pi = 0 if layer == 0 else 1
                fo = FO_L[layer]
                nc.vector.memset(acc_t[:], 0.0)
                acc3 = acc_t[:].rearrange("p (t f) -> p t f", f=F)

                for a, n, segs in plan["calls"]:
                    k = n // 128  # columns in this window
                    g = gatp.tile([128, (CHUNK // 128) * 128], f32, tag="g")
                    g3 = g[:, : k * 128].rearrange("p (c f) -> p c f", f=128)
                    nc.gpsimd.dma_gather(
                        out_ap=g3,
                        in_ap=src_ap(layer),
                        idxs_ap=gidx_t[pi][:, a // 16 : (a + n) // 16],
                        num_idxs=n,
                        num_idxs_reg=n,
                        elem_size=128,
                        single_packet=False,
                    )
                    mt = mskp.tile([128, (CHUNK // 128) * 128], f32, tag="m")
                    c0 = a // 128
                    nc.sync.dma_start(
                        mt[:, : k * 128], mask_in[pi][:, c0 * 128 : (c0 + k) * 128]
                    )
                    # mask-select in place
                    nc.vector.tensor_tensor(
                        out=g[:, : k * 128],
                        in0=g[:, : k * 128],
                        in1=mt[:, : k * 128],
                        op=mybir.AluOpType.mult,
                    )
                    for t, lo, hi in segs:
                        w = hi - lo
                        base = lo * 128
                        # fold columns (each 128 wide) down to one
                        while w > 1:
                            if w % 2 == 1:
                                nc.vector.tensor_tensor(
                                    out=g[:, base : base + 128],
                                    in0=g[:, base : base + 128],
                                    in1=g[:, base + (w - 1) * 128 : base + w * 128],
                                    op=mybir.AluOpType.add,
                                )
                                w -= 1
                            h = w // 2
                            nc.vector.tensor_tensor(
                                out=g[:, base : base + h * 128],
                                in0=g[:, base : base + h * 128],
                                in1=g[:, base + h * 128 : base + 2 * h * 128],
                                op=mybir.AluOpType.add,
                            )
                            w = h
                        # fold quad bands 128 -> 64 -> 32
                        nc.vector.tensor_tensor(
                            out=g[:, base : base + 64],
                            in0=g[:, base : base + 64],
                            in1=g[:, base + 64 : base + 128],
                            op=mybir.AluOpType.add,
                        )
                        nc.vector.tensor_tensor(
                            out=g[:, base : base + 32],
                            in0=g[:, base : base + 32],
                            in1=g[:, base + 32 : base + 64],
                            op=mybir.AluOpType.add,
                        )
                        nc.vector.tensor_tensor(
                            out=acc3[:, t, :],
                            in0=acc3[:, t, :],
                            in1=g[:, base : base + 32],
                            op=mybir.AluOpType.add,
                        )

                # dense stage: z = acc @ W + b (+ leaky relu)
                for q in range(NT):
                    at = zstp.tile([32, 128], f32, tag="aggT")
                    for k in range(4):
                        nc.vector.transpose(
                            at[:, 32 * k : 32 * k + 32], acc3[32 * k : 32 * k + 32, q, :]
                        )
                    pz = psump.tile([128, fo], f32, tag="pz")
                    nc.tensor.matmul(pz[:], lhsT=at[:], rhs=w_t[layer][:], start=True, stop=True)
                    zz = zstp.tile([128, fo], f32, tag="zz")
                    nc.vector.tensor_tensor(
                        out=zz[:], in0=pz[:], in1=b_t[layer][:], op=mybir.AluOpType.add
                    )
                    if layer < 2:
                        zm = zstp.tile([128, fo], f32, tag="zm")
                        nc.vector.tensor_scalar_mul(zm[:], zz[:], 0.1)
                        nc.vector.tensor_tensor(
                            out=zz[:], in0=zz[:], in1=zm[:], op=mybir.AluOpType.max
                        )
                        nc.sync.dma_start(
                            cc_in[layer][q * 128 : (q + 1) * 128, :], zz[:]
                        )
                    else:
                        nc.sync.dma_start(out[q * 128 : (q + 1) * 128, :], zz[:])

                if layer < 2:
                    nc.gpsimd.collective_compute(
                        "AllGather",
                        mybir.AluOpType.bypass,
                        ins=[cc_in[layer][:]],
                        outs=[cc_out[layer][:]],
                        replica_groups=[list(range(NC))],
                    )
    nc.compile()
    return nc


# ------------------------------------------------------------------- driver
def kernel(**inputs):
    _install_birpatch()
    x = np.asarray(inputs["x"], np.float32)
    src = np.asarray(inputs["src"], np.int64)
    dst = np.asarray(inputs["dst"], np.int64)
    Ws = [np.asarray(inputs[k], np.float32) for k in ("W1", "W2", "W3")]
    bs = [np.asarray(inputs[k], np.float32) for k in ("b1", "b2", "b3")]

    key = hash((src.tobytes(), dst.tobytes()))
    if key not in _cache:
        plan, per_core = _build_plan(src, dst)
        nc = _build_nc(plan)
        _cache[key] = (nc, plan, per_core)
    nc, plan, per_core = _cache[key]

    xqv = x.reshape(XQ, 128)

    in_maps = []
    for c in range(NC):
        pc = per_core[c]
        m = {
            "xq": xqv,
            "gx": pc["gx"],
            "gz": pc["gz"],
            "mx": pc["mx"],
            "mz": pc["mz"],
        }
        for i in range(3):
            m[f"w{i}"] = Ws[i]
            m[f"b{i}"] = np.tile(bs[i][None, :], (128, 1))
        in_maps.append(m)

    from concourse.bass_utils import run_bass_kernel_spmd

    trace = os.environ.get("GCN_TRACE") == "1"
    res = run_bass_kernel_spmd(nc, in_maps, core_ids=list(range(NC)), trace=trace)
    global last_exec_ns
    last_exec_ns = res.exec_time_ns

    out = np.zeros((N, FO_L[2]), np.float32)
    for c in range(NC):
        z = res.results[c]["out"]  # [S, 16] in position order
        out[c * OWN + per_core[c]["order"]] = z[:OWN]
    return out


# revision 5
# speedup vs baseline: 1.9858x; 1.1041x over previous
"""3-layer GCN on 8 TRN2 NeuronCores — scatter-free quad-gather design.

Each core owns 12500 dst nodes. Per layer, aggregation is a pure gather:
- dst slots are degree-sorted into 98 tiles of 128; tile t has D_t neighbor
  columns (shared across cores = max over cores).
- One dma_gather stream of 512B quad-row elements (4 feature rows per
  element; quad ids < 32768 fit int16 with no bank classes). Element for
  stream position p lands at [p%128, p//128] — position encodes
  (tile, column, node).
- A host-precomputed one-hot mask (streamed from DRAM, zero for pad cells,
  1.0 on byte-band src%4 for real edges) is multiplied in, then contiguous
  tree-folds reduce columns + quad bands to the [128, 32] aggregation tile.
- Dense stage: transpose + matmul + bias (+LeakyReLU) as usual; AllGather
  rebuilds the full node table between layers.

This removes all dma_scatter_add calls (previously ~45% of the serial
GpSimd descriptor-generation time) and all agg-table zeroing/RMW.
"""

import json
import os

import numpy as np

import concourse.bacc as bacc
import concourse.bass as bass
import concourse.mybir as mybir
import concourse.tile as tile

N = 100000
E_TOT = 1600000
NC = 8
OWN = 12500  # real dst nodes per core
S = 12544  # padded slots per core (98 * 128)
NT = S // 128  # 98 dst tiles
F = 32
FO_L = [32, 32, 16]
CHUNK = 2048  # edges per dma_gather call
XQ = 25000  # x quad rows (100000 / 4)
ZQ = (NC * S) // 4  # z quad rows

_cache = {}


# ---------------------------------------------------------------- BIR patch
def _split_sync_waits(bir_json, max_waits=1):
    d = json.loads(bir_json.decode() if isinstance(bir_json, (bytes, bytearray)) else bir_json)
    ctr = 0
    for f in d.get("functions", []):
        for bb in f.get("blocks", []):
            insts = bb.get("instructions", [])
            if not any(
                len((i.get("sync_info") or {}).get("on_wait") or []) > max_waits
                for i in insts
            ):
                continue
            out = []
            for inst in insts:
                si = inst.get("sync_info")
                waits = (si or {}).get("on_wait") or []
                if len(waits) > max_waits:
                    extra = waits[: len(waits) - max_waits]
                    si["on_wait"] = waits[len(waits) - max_waits :]
                    for w in extra:
                        ctr += 1
                        out.append(
                            {
                                "debug": inst.get("debug", 0),
                                "engine": inst["engine"],
                                "ins": [],
                                "outs": [],
                                "name": f"waitsplit-{ctr}",
                                "opcode": "NoOp",
                                "sync_info": {"on_update": [], "on_wait": [w]},
                            }
                        )
                out.append(inst)
            bb["instructions"] = out
    return json.dumps(d).encode()


_patched = False


def _install_birpatch():
    global _patched
    if _patched:
        return
    _patched = True
    import concourse.bass_utils as bu

    orig = bu.compile_bir_kernel

    def patched(bir_json, tmpdir, neff_name="file.neff"):
        return orig(_split_sync_waits(bir_json), tmpdir, neff_name=neff_name)

    bu.compile_bir_kernel = patched
    try:
        import concourse.bass2jax as b2j

        b2j.compile_bir_kernel = patched
    except ImportError:
        pass


# ------------------------------------------------------------- host planning
def _wrap_idx(arr):
    """[T] -> [128, T/16] int16, index i at [i%16 (replicated x8), i//16]."""
    T = arr.shape[0]
    w = arr.reshape(T // 16, 16).T  # [16, T/16]
    return np.tile(w, (8, 1)).copy()


def _rank_within_group(keys):
    """For each element, its occurrence rank among equal keys (keys arbitrary)."""
    order = np.argsort(keys, kind="stable")
    ks = keys[order]
    first = np.r_[True, ks[1:] != ks[:-1]]
    seg_start = np.flatnonzero(first)
    within = np.arange(len(ks)) - np.repeat(
        seg_start, np.diff(np.r_[seg_start, len(ks)])
    )
    rank = np.empty_like(within)
    rank[order] = within
    return rank


def _build_plan(src, dst):
    """Degree-sorted positional plan shared across cores + per-core arrays."""
    owner = dst // OWN
    # per-core degree sort
    orders = []  # core -> array [12500] node-local ids in position order
    pos_of = np.empty(N, np.int64)  # node -> global z-row position (core*S + pos)
    deg_sorted = np.zeros((NC, OWN), np.int64)
    for c in range(NC):
        m = owner == c
        dl = dst[m] - c * OWN
        deg = np.bincount(dl, minlength=OWN)
        order = np.argsort(-deg, kind="stable")
        orders.append(order)
        inv = np.empty(OWN, np.int64)
        inv[order] = np.arange(OWN)
        pos_of[c * OWN : (c + 1) * OWN] = c * S + inv
        deg_sorted[c] = deg[order]

    # shared per-tile column counts: max over cores of tile-max degree
    d_t = np.zeros(NT, np.int64)
    for t in range(NT):
        lo = t * 128
        hi = min(lo + 128, OWN)
        if lo >= OWN:
            break
        d_t[t] = max(int(deg_sorted[c][lo]) for c in range(NC))  # sorted desc
    colbase = np.zeros(NT + 1, np.int64)
    colbase[1:] = np.cumsum(d_t)
    ncols = int(colbase[NT])
    T = ncols * 128

    # static call/segment structure
    calls = []  # (start, n, [(tile, col_lo_local, col_hi_local), ...])
    a = 0
    while a < T:
        n = min(CHUNK, T - a)
        c0 = a // 128
        c1 = (a + n) // 128
        segs = []
        for t in range(NT):
            lo = max(colbase[t], c0)
            hi = min(colbase[t + 1], c1)
            if hi > lo:
                segs.append((t, int(lo - c0), int(hi - c0)))
        calls.append((int(a), int(n), segs))
        a += n

    # per-core index + mask arrays
    per_core = []
    zrow = pos_of  # node -> z-table row
    for c in range(NC):
        m = owner == c
        e_src = src[m]
        dl = dst[m] - c * OWN
        inv = np.empty(OWN, np.int64)
        inv[orders[c]] = np.arange(OWN)
        pos = inv[dl]  # 0..12499
        tl = pos // 128
        i = pos % 128
        j = _rank_within_group(pos)
        p_e = (colbase[tl] + j) * 128 + i
        assert (j < d_t[tl]).all()

        gx = np.zeros(T, np.int16)
        gz = np.zeros(T, np.int16)
        gx[p_e] = (e_src // 4).astype(np.int16)
        zr = zrow[e_src]
        gz[p_e] = (zr // 4).astype(np.int16)

        mx = np.zeros((T, 128), np.float32)
        mz = np.zeros((T, 128), np.float32)
        colx = (32 * (e_src % 4))[:, None] + np.arange(32)[None, :]
        colz = (32 * (zr % 4))[:, None] + np.arange(32)[None, :]
        mx[p_e[:, None], colx] = 1.0
        mz[p_e[:, None], colz] = 1.0
        # reshape to [128, ncols*128]: partition = stream i, free = (col, band)
        mx = mx.reshape(ncols, 128, 128).transpose(1, 0, 2).reshape(128, -1).copy()
        mz = mz.reshape(ncols, 128, 128).transpose(1, 0, 2).reshape(128, -1).copy()
        per_core.append(
            {
                "gx": _wrap_idx(gx),
                "gz": _wrap_idx(gz),
                "mx": mx,
                "mz": mz,
                "order": orders[c],
            }
        )
    return {"T": T, "ncols": ncols, "calls": calls, "colbase": colbase}, per_core


# --------------------------------------------------------------- bass build
def _build_nc(plan):
    nc = bacc.Bacc("TRN2", target_bir_lowering=False, debug=False, num_devices=NC)
    f32, i16 = mybir.dt.float32, mybir.dt.int16
    T, ncols = plan["T"], plan["ncols"]

    xq = nc.dram_tensor("xq", [XQ, 128], f32, kind="ExternalInput")
    w_in = [
        nc.dram_tensor(f"w{i}", [F, FO_L[i]], f32, kind="ExternalInput")
        for i in range(3)
    ]
    b_in = [
        nc.dram_tensor(f"b{i}", [128, FO_L[i]], f32, kind="ExternalInput")
        for i in range(3)
    ]
    gidx_in = [
        nc.dram_tensor("gx", [128, T // 16], i16, kind="ExternalInput"),
        nc.dram_tensor("gz", [128, T // 16], i16, kind="ExternalInput"),
    ]
    mask_in = [
        nc.dram_tensor("mx", [128, ncols * 128], f32, kind="ExternalInput"),
        nc.dram_tensor("mz", [128, ncols * 128], f32, kind="ExternalInput"),
    ]
    out = nc.dram_tensor("out", [S, FO_L[2]], f32, kind="ExternalOutput")

    cc_in = [
        nc.dram_tensor(f"cc_in{i}", [S, F], f32, kind="Internal") for i in range(2)
    ]
    cc_out = [
        nc.dram_tensor(f"cc_out{i}", [NC * S, F], f32, kind="Internal", addr_space="Shared")
        for i in range(2)
    ]

    def src_ap(layer):
        if layer == 0:
            return bass.AP(xq[:].tensor, 0, [[128, XQ], [1, 128]])
        t = cc_out[layer - 1][:].tensor
        return bass.AP(t, 0, [[128, ZQ], [1, 128]])

    with tile.TileContext(nc) as tc:
        with (
            tc.tile_pool(name="consts", bufs=1) as constp,
            tc.tile_pool(name="idx", bufs=1) as idxp,
            tc.tile_pool(name="acc", bufs=1) as accp,
            tc.tile_pool(name="gat", bufs=3) as gatp,
            tc.tile_pool(name="msk", bufs=3) as mskp,
            tc.tile_pool(name="zst", bufs=4) as zstp,
            tc.tile_pool(name="psum", bufs=4, space="PSUM") as psump,
        ):
            w_t = []
            b_t = []
            for i in range(3):
                wt = constp.tile([F, FO_L[i]], f32, tag=f"w{i}")
                bt = constp.tile([128, FO_L[i]], f32, tag=f"b{i}")
                nc.sync.dma_start(wt[:], w_in[i][:])
                nc.sync.dma_start(bt[:], b_in[i][:])
                w_t.append(wt)
                b_t.append(bt)

            gidx_t = [
                idxp.tile([128, T // 16], i16, tag="gx", name="gx_t"),
                idxp.tile([128, T // 16], i16, tag="gz", name="gz_t"),
            ]
            nc.sync.dma_start(gidx_t[0][:], gidx_in[0][:])
            nc.sync.dma_start(gidx_t[1][:], gidx_in[1][:])

            acc_t = accp.tile([128, NT * F], f32, tag="acc")

            last_call = {}
            for k, (a, n, segs) in enumerate(plan["calls"]):
                for t, _, _ in segs:
                    last_call[t] = k

            def dense_tile(layer, q):
                fo = FO_L[layer]
                acc3 = acc_t[:].rearrange("p (t f) -> p t f", f=F)
                at = zstp.tile([32, 128], f32, tag="aggT", name="at")
                for k in range(4):
                    nc.vector.transpose(
                        at[:, 32 * k : 32 * k + 32], acc3[32 * k : 32 * k + 32, q, :]
                    )
                pz = psump.tile([128, fo], f32, tag="pz", name="pz")
                nc.tensor.matmul(pz[:], lhsT=at[:], rhs=w_t[layer][:], start=True, stop=True)
                zz = zstp.tile([128, fo], f32, tag="zz", name="zz")
                nc.vector.tensor_tensor(
                    out=zz[:], in0=pz[:], in1=b_t[layer][:], op=mybir.AluOpType.add
                )
                if layer < 2:
                    zm = zstp.tile([128, fo], f32, tag="zm", name="zm")
                    nc.vector.tensor_scalar_mul(zm[:], zz[:], 0.1)
                    nc.vector.tensor_tensor(
                        out=zz[:], in0=zz[:], in1=zm[:], op=mybir.AluOpType.max
                    )
                    nc.sync.dma_start(cc_in[layer][q * 128 : (q + 1) * 128, :], zz[:])
                else:
                    nc.sync.dma_start(out[q * 128 : (q + 1) * 128, :], zz[:])

            for layer in range(3):
                pi = 0 if layer == 0 else 1
                fo = FO_L[layer]
                nc.vector.memset(acc_t[:], 0.0)
                acc3 = acc_t[:].rearrange("p (t f) -> p t f", f=F)

                for ci, (a, n, segs) in enumerate(plan["calls"]):
                    k = n // 128  # columns in this window
                    g = gatp.tile([128, (CHUNK // 128) * 128], f32, tag="g")
                    g3 = g[:, : k * 128].rearrange("p (c f) -> p c f", f=128)
                    nc.gpsimd.dma_gather(
                        out_ap=g3,
                        in_ap=src_ap(layer),
                        idxs_ap=gidx_t[pi][:, a // 16 : (a + n) // 16],
                        num_idxs=n,
                        num_idxs_reg=n,
                        elem_size=128,
                        single_packet=False,
                    )
                    mt = mskp.tile([128, (CHUNK // 128) * 128], f32, tag="m")
                    c0 = a // 128
                    nc.sync.dma_start(
                        mt[:, : k * 128], mask_in[pi][:, c0 * 128 : (c0 + k) * 128]
                    )
                    # mask-select in place
                    nc.vector.tensor_tensor(
                        out=g[:, : k * 128],
                        in0=g[:, : k * 128],
                        in1=mt[:, : k * 128],
                        op=mybir.AluOpType.mult,
                    )
                    for t, lo, hi in segs:
                        w = hi - lo
                        base = lo * 128
                        # fold columns (each 128 wide) down to one
                        while w > 1:
                            if w % 2 == 1:
                                nc.vector.tensor_tensor(
                                    out=g[:, base : base + 128],
                                    in0=g[:, base : base + 128],
                                    in1=g[:, base + (w - 1) * 128 : base + w * 128],
                                    op=mybir.AluOpType.add,
                                )
                                w -= 1
                            h = w // 2
                            nc.vector.tensor_tensor(
                                out=g[:, base : base + h * 128],
                                in0=g[:, base : base + h * 128],
                                in1=g[:, base + h * 128 : base + 2 * h * 128],
                                op=mybir.AluOpType.add,
                            )
                            w = h
                        # fold quad bands 128 -> 64 -> 32
                        nc.vector.tensor_tensor(
                            out=g[:, base : base + 64],
                            in0=g[:, base : base + 64],
                            in1=g[:, base + 64 : base + 128],
                            op=mybir.AluOpType.add,
                        )
                        nc.vector.tensor_tensor(
                            out=g[:, base : base + 32],
                            in0=g[:, base : base + 32],
                            in1=g[:, base + 32 : base + 64],
                            op=mybir.AluOpType.add,
                        )
                        nc.vector.tensor_tensor(
                            out=acc3[:, t, :],
                            in0=acc3[:, t, :],
                            in1=g[:, base : base + 32],
                            op=mybir.AluOpType.add,
                        )
                    # dense stage for tiles whose aggregation just completed
                    for t, _, _ in segs:
                        if last_call[t] == ci:
                            dense_tile(layer, t)
                # tiles with zero columns never appear in segs
                for t in range(NT):
                    if t not in last_call:
                        dense_tile(layer, t)

                if layer < 2:
                    nc.gpsimd.collective_compute(
                        "AllGather",
                        mybir.AluOpType.bypass,
                        ins=[cc_in[layer][:]],
                        outs=[cc_out[layer][:]],
                        replica_groups=[list(range(NC))],
                    )
    nc.compile()
    return nc


# ------------------------------------------------------------------- driver
def kernel(**inputs):
    _install_birpatch()
    x = np.asarray(inputs["x"], np.float32)
    src = np.asarray(inputs["src"], np.int64)
    dst = np.asarray(inputs["dst"], np.int64)
    Ws = [np.asarray(inputs[k], np.float32) for k in ("W1", "W2", "W3")]
    bs = [np.asarray(inputs[k], np.float32) for k in ("b1", "b2", "b3")]

    key = hash((src.tobytes(), dst.tobytes()))
    if key not in _cache:
        plan, per_core = _build_plan(src, dst)
        nc = _build_nc(plan)
        _cache[key] = (nc, plan, per_core)
    nc, plan, per_core = _cache[key]

    xqv = x.reshape(XQ, 128)

    in_maps = []
    for c in range(NC):
        pc = per_core[c]
        m = {
            "xq": xqv,
            "gx": pc["gx"],
            "gz": pc["gz"],
            "mx": pc["mx"],
            "mz": pc["mz"],
        }
        for i in range(3):
            m[f"w{i}"] = Ws[i]
            m[f"b{i}"] = np.tile(bs[i][None, :], (128, 1))
        in_maps.append(m)

    from concourse.bass_utils import run_bass_kernel_spmd

    trace = os.environ.get("GCN_TRACE") == "1"
    res = run_bass_kernel_spmd(nc, in_maps, core_ids=list(range(NC)), trace=trace)
    global last_exec_ns
    last_exec_ns = res.exec_time_ns

    out = np.zeros((N, FO_L[2]), np.float32)
    for c in range(NC):
        z = res.results[c]["out"]  # [S, 16] in position order
        out[c * OWN + per_core[c]["order"]] = z[:OWN]
    return out
